# revision 1
# baseline (speedup 1.0000x reference)
"""Trainium2 Bass kernel for the 2-stack GRU decoder with 5-wide sliding
window attention (nn_DEC_59880434041064).

Strategy: pure data parallel over batch (1024 -> 8 cores x 128).
Per-core layout keeps features on partitions and batch on the free dim so
the sequential GRU needs no transposes.  The attention + output projection
is algebraically collapsed on the host: only four per-(b,t) scalar fields
(window score s_p, q = u_c.r, p = u_r.r, g = w_o.r) are needed, computed by
an N=4 matmul against each fresh hidden state, so no [H,B,T] context GEMM
ever runs on device.
"""

import os

import numpy as np

import concourse.bass as bass
import concourse.mybir as mybir
import concourse.tile as tile
from concourse import bacc
from concourse.bass_utils import run_bass_kernel_spmd

FP = mybir.dt.float32
AL = mybir.AluOpType
AF = mybir.ActivationFunctionType

B, L, H, NIN = 1024, 512, 128, 2
L = int(os.environ.get("BASS_GRU_L", L))  # debug-size override
NCORES = 8
BL = B // NCORES            # 128 samples per core
ATTN, DLY = 5, 10
WIN = L - ATTN              # 507 valid attention positions (t >= 5)
XCH = 64                    # timesteps per x-chunk DMA
SCH = 64                    # timesteps per score-psum chunk

# weight blob column offsets (blob is [128, NBLOB] fp32)
HH_OFF = 0                  # 4 cells x 384
IH1_OFF = HH_OFF + 4 * 384  # 2 streams x 384 (layer-1 input weights)
IH0_OFF = IH1_OFF + 2 * 384  # 2 streams x 384 (layer-0 aug weights, rows 0..2)
BR_OFF = IH0_OFF + 2 * 384  # bias rows (row 0): 10 x 128
SC_OFF = BR_OFF + 10 * 128  # score vectors: 2 streams x 4 cols
CONST_OFF = SC_OFF + 8      # per-stream additive consts (bcast on partitions)
BCOL_OFF = CONST_OFF + 2    # per-partition bias columns: r1(2) z1(2) in1(2) hn(4)
NBLOB = BCOL_OFF + 10

_BUILT = {}


def _cell(s, l):
    return s * 2 + l


def _bias_row_off(kind, s, l=None):
    # order: l1 r (2), l1 z (2), l1 in (2), hn for 4 cells
    if kind == "r1":
        return BR_OFF + s * 128
    if kind == "z1":
        return BR_OFF + (2 + s) * 128
    if kind == "in1":
        return BR_OFF + (4 + s) * 128
    if kind == "hn":
        return BR_OFF + (6 + _cell(s, l)) * 128
    raise KeyError(kind)


def _emit(tc, recv, wb, onesd, out_ap, scr):
    nc = tc.nc
    import contextlib

    outer = contextlib.ExitStack()
    const = outer.enter_context(tc.tile_pool(name="const", bufs=1))
    wsb = const.tile([128, NBLOB], FP)
    nc.sync.dma_start(wsb[:], wb[:])
    ones = const.tile([1, 128], FP)
    nc.vector.memset(ones[:], 1.0)
    wsb_post = wsb

    with contextlib.ExitStack() as ctx:
        xpool = ctx.enter_context(tc.tile_pool(name="xch", bufs=2))
        hpool = ctx.enter_context(tc.tile_pool(name="h", bufs=4))
        pw = ctx.enter_context(tc.tile_pool(name="pw", bufs=4))
        prz = [
            ctx.enter_context(tc.tile_pool(name=f"prz{l}", bufs=2, space="PSUM"))
            for l in range(2)
        ]
        pn = [
            ctx.enter_context(tc.tile_pool(name=f"pn{l}", bufs=1, space="PSUM"))
            for l in range(2)
        ]
        psc = ctx.enter_context(tc.tile_pool(name="psc", bufs=2, space="PSUM"))

        def whh(s, l):  # [128, 384] lhsT
            o = HH_OFF + _cell(s, l) * 384
            return wsb[:, o:o + 384]

        def wih1(s):
            o = IH1_OFF + s * 384
            return wsb[:, o:o + 384]

        def wih0(s):  # [3, 384] augmented lhsT
            o = IH0_OFF + s * 384
            return wsb[0:3, o:o + 384]

        def brow(kind, s, l=None):  # [1, 128]
            o = _bias_row_off(kind, s, l)
            return wsb[0:1, o:o + 128]

        def scw(s):  # [128, 4]
            o = SC_OFF + s * 4
            return wsb[:, o:o + 4]

        def bcol(kind, s, l=None):  # [128, 1] per-partition bias column
            idx = {"r1": 0, "z1": 2, "in1": 4}.get(kind)
            if idx is None:
                idx = 6 + _cell(s, l)
            else:
                idx += s
            o = BCOL_OFF + idx
            return wsb[:, o:o + 1]

        # initial hidden state (per layer, both streams concatenated)
        h = []
        for l in range(2):
            t0 = hpool.tile([128, 256], FP, tag=f"h{l}")
            nc.vector.memset(t0[:], 0.0)
            h.append(t0)

        def gru_pair(l, h_in, mm_rhs):
            """Emit one layer-pair GRU step. mm_rhs: per-stream extra input
            rhs ([3,128] aug x for l0, [128,128] r_l0 for l1). Returns h'."""
            rz = prz[l].tile([128, 512], FP, name=f"rz{l}")
            np_ = pn[l].tile([128, 512], FP, name=f"npm{l}")
            # emission order = PE program order; r-gate (both streams) first
            # so the sigmoid unblocks after 4 matmuls, then hn/in (feed the
            # tt/uu chain), z last (its sigmoid is off the critical path).
            def cellw(s):
                hs = h_in[:, s * 128:(s + 1) * 128]
                W = whh(s, l)
                wih = wih0(s) if l == 0 else wih1(s)
                return hs, W, wih, mm_rhs[s]

            for s in range(2):
                hs, W, wih, xr = cellw(s)
                rcol = s * 128
                nc.tensor.matmul(rz[:, rcol:rcol + 128], W[:, 0:128], hs,
                                 start=True, stop=False)
                nc.tensor.matmul(rz[:, rcol:rcol + 128], wih[:, 0:128], xr,
                                 start=False, stop=True)
            for s in range(2):
                hs, W, wih, xr = cellw(s)
                ncol = s * 128
                nc.tensor.matmul(np_[:, ncol:ncol + 128], W[:, 256:384], hs,
                                 start=True, stop=True)
            for s in range(2):
                hs, W, wih, xr = cellw(s)
                zcol = 256 + s * 128
                nc.tensor.matmul(rz[:, zcol:zcol + 128], W[:, 128:256], hs,
                                 start=True, stop=False)
                nc.tensor.matmul(rz[:, zcol:zcol + 128], wih[:, 128:256], xr,
                                 start=False, stop=True)
            for s in range(2):
                hs, W, wih, xr = cellw(s)
                icol = 256 + s * 128
                nc.tensor.matmul(np_[:, icol:icol + 128], wih[:, 256:384], xr,
                                 start=True, stop=True)

            # pointwise; rz psum holds [r1|r2|z1'|z2'].  Layer-1 biases are
            # folded via per-partition ACT bias columns; layer-0 rz biases
            # ride in the augmented x row.  hn/in biases fold into the STTs.
            rsb = pw.tile([128, 256], FP, tag=f"rsb{l}", name=f"rsb{l}")
            zsb = pw.tile([128, 256], FP, tag=f"zsb{l}", name=f"zsb{l}")
            if l == 0:
                for s in range(2):
                    nc.scalar.activation(rsb[:, s * 128:(s + 1) * 128],
                                         rz[:, s * 128:s * 128 + 128],
                                         AF.Sigmoid)
                nc.scalar.activation(zsb[:], rz[:, 256:512], AF.Sigmoid)
            else:
                for s in range(2):
                    nc.scalar.activation(rsb[:, s * 128:(s + 1) * 128],
                                         rz[:, s * 128:s * 128 + 128],
                                         AF.Sigmoid, bias=bcol("r1", s))
                for s in range(2):
                    nc.scalar.activation(zsb[:, s * 128:(s + 1) * 128],
                                         rz[:, 256 + s * 128:384 + s * 128],
                                         AF.Sigmoid, bias=bcol("z1", s))
            tt = pw.tile([128, 256], FP, tag=f"tt{l}", name=f"tt{l}")
            uu = pw.tile([128, 256], FP, tag=f"uu{l}", name=f"uu{l}")
            nn_ = pw.tile([128, 256], FP, tag=f"nn{l}", name=f"nn{l}")
            zhn = pw.tile([128, 256], FP, tag=f"zhn{l}", name=f"zhn{l}")
            zn = pw.tile([128, 256], FP, tag=f"zn{l}", name=f"zn{l}")
            hn_t = hpool.tile([128, 256], FP, tag=f"h{l}", name=f"hn{l}")
            for s in range(2):
                c0, c1 = s * 128, (s + 1) * 128
                nc.vector.scalar_tensor_tensor(
                    tt[:, c0:c1], np_[:, c0:c1], bcol("hn", s, l),
                    rsb[:, c0:c1], AL.add, AL.mult)
                if l == 0:
                    nc.vector.tensor_add(uu[:, c0:c1],
                                         np_[:, 256 + c0:256 + c1],
                                         tt[:, c0:c1])
                else:
                    nc.vector.scalar_tensor_tensor(
                        uu[:, c0:c1], np_[:, 256 + c0:256 + c1],
                        bcol("in1", s), tt[:, c0:c1], AL.add, AL.add)
                nc.scalar.activation(nn_[:, c0:c1], uu[:, c0:c1], AF.Tanh)
                # h' = z'*n - (z'-1)*h ; zhn off the critical path
                nc.vector.scalar_tensor_tensor(
                    zhn[:, c0:c1], zsb[:, c0:c1], 1.0, h_in[:, c0:c1],
                    AL.subtract, AL.mult)
                nc.vector.tensor_mul(zn[:, c0:c1], zsb[:, c0:c1],
                                     nn_[:, c0:c1])
                nc.vector.tensor_sub(hn_t[:, c0:c1], zn[:, c0:c1],
                                     zhn[:, c0:c1])
            return hn_t

        # wavefront: iteration tau emits l0(tau) and, one step behind, the
        # l1 pair for tau-1 -- the skew keeps the two recurrence chains
        # decoupled in every engine's in-order queue.
        xch = None
        scp = None
        h0_prev = None  # l0 output of the previous step (l1's input)
        for tau in range(L + 1):
            if tau < L:
                t = tau
                if t % XCH == 0:
                    xch = xpool.tile([3, XCH * 128], FP, tag="xch", name=f"xch{t}")
                    # recv is host-transposed to [i, t, b]; chunk contiguous
                    src = recv[:, t:t + XCH, :]
                    dst = xch[0:2, :].rearrange("i (t b) -> i t b", b=128)
                    nc.sync.dma_start(dst, src)
                    nc.sync.dma_start(xch[2:3, :], onesd[:, 0:XCH * 128])
                toff = t % XCH
                x_t = xch[:, toff * 128:(toff + 1) * 128]
                h0_new = gru_pair(0, h[0], [x_t, x_t])
            if tau >= 1:
                u = tau - 1
                if u % SCH == 0:
                    scp = psc.tile([128, SCH * 8], FP, tag="scp", name=f"scp{u}")
                rl0 = [h0_prev[:, 0:128], h0_prev[:, 128:256]]
                h1_new = gru_pair(1, h[1], rl0)
                so = (u % SCH) * 8
                for s in range(2):
                    nc.tensor.matmul(scp[:, so + s * 4:so + s * 4 + 4],
                                     h1_new[:, s * 128:(s + 1) * 128], scw(s),
                                     start=True, stop=True)
                if (u + 1) % SCH == 0:
                    stg = pw.tile([128, SCH * 8], FP, tag="scstg",
                                  name=f"stg{u}")
                    nc.vector.tensor_copy(stg[:], scp[:])
                    dst = scr[:, u + 1 - SCH:u + 1, :]
                    nc.sync.dma_start(
                        dst, stg[:].rearrange("b (t q) -> b t q", q=8))
                h[1] = h1_new
            if tau < L:
                h0_prev = h0_new
                h[0] = h0_new

    # ---- post phase: window softmax + output assembly ----
    with contextlib.ExitStack() as ctx:
        fp = ctx.enter_context(tc.tile_pool(name="fields", bufs=1))
        tp = ctx.enter_context(tc.tile_pool(name="ptmp", bufs=2))
        osb = []
        for s in range(2):
            fld = []
            for q in range(4):
                f = fp.tile([128, L], FP, tag=f"f{s}{q}")
                src = scr[:, :, s * 4 + q:s * 4 + q + 1].rearrange(
                    "b t o -> b (t o)")
                # split so merged (b t) dim stays under the 16-bit ISA field
                nc.sync.dma_start(f[0:64, :], src[0:64, :])
                nc.sync.dma_start(f[64:128, :], src[64:128, :])
                fld.append(f)
            sp, qf, pf, gf = fld

            def v(x, j):
                return x[:, j:j + WIN]

            m1 = tp.tile([128, WIN], FP, tag="m1")
            nc.vector.tensor_max(m1[:], v(sp, 0), v(sp, 1))
            m2 = tp.tile([128, WIN], FP, tag="m2")
            nc.vector.tensor_max(m2[:], v(sp, 2), v(sp, 3))
            m3 = tp.tile([128, WIN], FP, tag="m3")
            nc.vector.tensor_max(m3[:], m1[:], m2[:])
            mx = tp.tile([128, WIN], FP, tag="mx")
            nc.vector.tensor_max(mx[:], m3[:], v(sp, 4))
            es = []
            for j in range(ATTN):
                d = tp.tile([128, WIN], FP, tag=f"d{j}")
                nc.vector.tensor_sub(d[:], v(sp, j), mx[:])
                e = tp.tile([128, WIN], FP, tag=f"e{j}")
                nc.scalar.activation(e[:], d[:], AF.Exp)
                es.append(e)
            d01 = tp.tile([128, WIN], FP, tag="d01")
            nc.vector.tensor_add(d01[:], es[0][:], es[1][:])
            d23 = tp.tile([128, WIN], FP, tag="d23")
            nc.vector.tensor_add(d23[:], es[2][:], es[3][:])
            d03 = tp.tile([128, WIN], FP, tag="d03")
            nc.vector.tensor_add(d03[:], d01[:], d23[:])
            den = tp.tile([128, WIN], FP, tag="den")
            nc.vector.tensor_add(den[:], d03[:], es[4][:])
            # numerator: sum_j e_j * q(t-4+j)
            nums = []
            for j in range(ATTN):
                nmj = tp.tile([128, WIN], FP, tag=f"nm{j}")
                nc.gpsimd.tensor_mul(nmj[:], es[j][:], v(qf, j))
                nums.append(nmj)
            n01 = tp.tile([128, WIN], FP, tag="n01")
            nc.gpsimd.tensor_add(n01[:], nums[0][:], nums[1][:])
            n23 = tp.tile([128, WIN], FP, tag="n23")
            nc.gpsimd.tensor_add(n23[:], nums[2][:], nums[3][:])
            n03 = tp.tile([128, WIN], FP, tag="n03")
            nc.vector.tensor_add(n03[:], n01[:], n23[:])
            num = tp.tile([128, WIN], FP, tag="num")
            nc.vector.tensor_add(num[:], n03[:], nums[4][:])
            rec = tp.tile([128, WIN], FP, tag="rec")
            nc.vector.reciprocal(rec[:], den[:])
            att = tp.tile([128, WIN], FP, tag="att")
            nc.vector.tensor_mul(att[:], num[:], rec[:])
            ot = fp.tile([128, L], FP, tag=f"o{s}")
            # o = att + p + C_s  (C_s per-stream additive constant from blob)
            cs = wsb_post[:, CONST_OFF + s:CONST_OFF + s + 1]
            nc.vector.scalar_tensor_tensor(
                ot[:, ATTN:L], att[:], cs, v(pf, ATTN), AL.add, AL.add)
            if s == 0:
                # passthrough region t<ATTN uses g = w_o_h . r1 (no b_c term)
                nc.vector.tensor_scalar_add(
                    ot[:, 0:ATTN], gf[:, 0:ATTN], 0.0)
            osb.append(ot)

        dec = fp.tile([128, L], FP, tag="dec")
        nc.vector.tensor_add(dec[:, 0:L - DLY], osb[0][:, 0:L - DLY],
                             osb[1][:, DLY:L])
        for i in range(DLY):
            c = L - DLY + i
            nc.vector.tensor_add(dec[:, c:c + 1], osb[0][:, c:c + 1],
                                 osb[1][:, L - 1:L])
        # u_b1 correction for passthrough cols is baked on host into g field
        sig = fp.tile([128, L], FP, tag="sig")
        nc.scalar.activation(sig[:], dec[:], AF.Sigmoid)
        nc.sync.dma_start(out_ap[:], sig[:])
    outer.close()


def _build(nc_count=NCORES):
    key = nc_count
    if key in _BUILT:
        return _BUILT[key]
    nc = bacc.Bacc("TRN2", target_bir_lowering=False, debug=False,
                   num_devices=nc_count)
    recv = nc.dram_tensor("recv", [NIN, L, BL], FP, kind="ExternalInput").ap()
    wb = nc.dram_tensor("wblob", [128, NBLOB], FP, kind="ExternalInput").ap()
    onesd = nc.dram_tensor("onesd", [1, XCH * 128], FP,
                           kind="ExternalInput").ap()
    out_ap = nc.dram_tensor("out", [BL, L], FP, kind="ExternalOutput").ap()
    scr = nc.dram_tensor("scores", [128, L, 8], FP).ap()
    with tile.TileContext(nc) as tc:
        _emit(tc, recv, wb, onesd, out_ap, scr)
    nc.compile()
    _BUILT[key] = nc
    return nc


def _prep_weights(inp):
    """Host-side packing of all weights into the [128, NBLOB] blob."""
    wb = np.zeros((128, NBLOB), np.float32)

    def neg_z(m):  # m: [384, ...]; negate z gate rows
        m = m.copy()
        m[128:256] = -m[128:256]
        return m

    for s in range(2):
        sn = s + 1
        for l in range(2):
            whh = neg_z(np.asarray(inp[f"w_hh{sn}_l{l}"], np.float32))
            wb[:, HH_OFF + _cell(s, l) * 384:HH_OFF + _cell(s, l) * 384 + 384] = \
                whh.T  # [128, 384]
            b_hh = np.asarray(inp[f"b_hh{sn}_l{l}"], np.float32)
            o = _bias_row_off("hn", s, l)
            wb[0, o:o + 128] = b_hh[256:384]
        # layer 1 plain lhsT
        wih1 = neg_z(np.asarray(inp[f"w_ih{sn}_l1"], np.float32))
        wb[:, IH1_OFF + s * 384:IH1_OFF + s * 384 + 384] = wih1.T
        b_ih1 = np.asarray(inp[f"b_ih{sn}_l1"], np.float32)
        b_hh1 = np.asarray(inp[f"b_hh{sn}_l1"], np.float32)
        o = _bias_row_off("r1", s)
        wb[0, o:o + 128] = b_ih1[0:128] + b_hh1[0:128]
        o = _bias_row_off("z1", s)
        wb[0, o:o + 128] = -(b_ih1[128:256] + b_hh1[128:256])
        o = _bias_row_off("in1", s)
        wb[0, o:o + 128] = b_ih1[256:384]
        # layer 0 augmented [3, 384]
        wih0 = neg_z(np.asarray(inp[f"w_ih{sn}_l0"], np.float32))
        b_ih0 = np.asarray(inp[f"b_ih{sn}_l0"], np.float32)
        b_hh0 = np.asarray(inp[f"b_hh{sn}_l0"], np.float32)
        aug = np.zeros((3, 384), np.float32)
        aug[0:2] = wih0.T
        aug[2, 0:128] = b_ih0[0:128] + b_hh0[0:128]
        aug[2, 128:256] = -(b_ih0[128:256] + b_hh0[128:256])
        aug[2, 256:384] = b_ih0[256:384]
        wb[:, IH0_OFF + s * 384:IH0_OFF + s * 384 + 384] = 0.0
        wb[0:3, IH0_OFF + s * 384:IH0_OFF + s * 384 + 384] = aug

    # score vectors
    w_a = np.asarray(inp["w_a"], np.float32)
    w_c = np.asarray(inp["w_c"], np.float32)
    b_c = np.asarray(inp["b_c"], np.float32)
    w_o = np.asarray(inp["w_o"], np.float32)
    b_o = float(np.asarray(inp["b_o"], np.float32)[0])
    consts = {}
    for s in range(2):
        wo_s = w_o[0, s * 128:(s + 1) * 128]
        u_c = wo_s @ w_c[:, 0:128]
        u_r = wo_s @ w_c[:, 128:256]
        u_b = float(wo_s @ b_c)
        sc = np.stack([w_a[0, 128:256], u_c, u_r, wo_s], axis=1)  # [128, 4]
        wb[:, SC_OFF + s * 4:SC_OFF + s * 4 + 4] = sc
        consts[f"u_b{s}"] = u_b
    consts["b_o"] = b_o
    # per-stream additive constants, broadcast down the partition dim:
    # stream 0 carries u_b1; stream 1 carries u_b2 + b_o.
    wb[:, CONST_OFF + 0] = consts["u_b0"]
    wb[:, CONST_OFF + 1] = consts["u_b1"] + b_o
    # per-partition bias columns (r1, z1, in1 per stream; hn per cell)
    for s in range(2):
        sn = s + 1
        b_ih1 = np.asarray(inp[f"b_ih{sn}_l1"], np.float32)
        b_hh1 = np.asarray(inp[f"b_hh{sn}_l1"], np.float32)
        wb[:, BCOL_OFF + 0 + s] = b_ih1[0:128] + b_hh1[0:128]
        wb[:, BCOL_OFF + 2 + s] = -(b_ih1[128:256] + b_hh1[128:256])
        wb[:, BCOL_OFF + 4 + s] = b_ih1[256:384]
        for l in range(2):
            b_hh = np.asarray(inp[f"b_hh{sn}_l{l}"], np.float32)
            wb[:, BCOL_OFF + 6 + _cell(s, l)] = b_hh[256:384]
    return wb, consts


def kernel(**inputs):
    recv = np.ascontiguousarray(np.asarray(inputs["received"], np.float32))
    assert recv.shape == (B, L, NIN)
    assert int(inputs.get("attn_num", ATTN)) == ATTN
    assert int(inputs.get("d_delay", DLY)) == DLY
    wb, _ = _prep_weights(inputs)
    nc = _build()
    core_ids = list(range(NCORES))
    ones = np.ones((1, XCH * 128), np.float32)
    in_maps = []
    for c in range(NCORES):
        shard = recv[c * BL:(c + 1) * BL]  # [BL, L, 2]
        in_maps.append({
            "recv": np.ascontiguousarray(shard.transpose(2, 1, 0)),
            "wblob": wb,
            "onesd": ones,
        })
    kw = {}
    if os.environ.get("BASS_GRU_TRACE"):
        kw = dict(trace=True, tmpdir=os.environ.get("BASS_GRU_TRACE_DIR",
                                                    "/tmp/gru_trace"))
    res = run_bass_kernel_spmd(nc, in_maps, core_ids, **kw)
    if getattr(res, "exec_time_ns", None) is not None:
        print(f"HW exec time: {res.exec_time_ns} ns", flush=True)
    outs = [res.results[i]["out"] for i in range(NCORES)]
    dec = np.concatenate(outs, axis=0)  # [1024, 512] -- sigmoid(dec_pre)
    return dec[..., None].astype(np.float32)

